# revision 7
# baseline (speedup 1.0000x reference)
"""CfC (closed-form continuous-time) RNN kernel for Trainium2, 8 NeuronCores.

Problem: B=256, L=1024, C=128, U=BBU=256.
    step: z = 1.7159*tanh(0.666*(concat(x_t,h) @ Wbb + bbb))
          ff1 = tanh(z@Wff1+bff1); ff2 = tanh(z@Wff2+bff2)
          s = sigmoid(z@(Wtb-Wta) + (btb-bta))
          h = ff1 + s*(ff2-ff1)     (emitted every step)

Strategy: data-parallel over batch (32 per core). Everything feature-major:
features on SBUF partitions, batch on the free dim. Weights stay SBUF-resident
as bf16 128x128 stationary tiles (FWL); activations stream as the bf16 moving
operand (N=32). Biases are injected into PSUM with tiny K=2/K=4 matmuls against
constant selector matrices so the ACT/DVE ops need no per-slice bias handling.
Gating math runs fp32 on ACT/DVE; h is rounded to bf16 once per step (feeds
both the recurrence and the output DMA).
"""

import numpy as np
import ml_dtypes

import concourse.bass as bass
import concourse.mybir as mybir
import concourse.tile as tile
from concourse import bacc, bass_utils
from concourse._compat import with_exitstack

BF16 = ml_dtypes.bfloat16
AF = mybir.ActivationFunctionType

B, L, C, U, BBU = 256, 1024, 128, 256, 256
N_CORES = 8
BC = B // N_CORES  # 32 batch per core
H2 = 2 * BC        # 64: [u0|u1] concatenated along free dim


def build_nc(seq_len=L, chunk=256, out_piece=32):
    """Build the Bass/Tile program (same NEFF for all 8 cores)."""
    out_piece = min(out_piece, chunk)
    nc = bacc.Bacc("TRN2", target_bir_lowering=False, debug=False,
                   num_devices=N_CORES)
    dt = mybir.dt

    xT = nc.dram_tensor("xT", [C, seq_len, BC], dt.bfloat16, kind="ExternalInput").ap()
    h0T = nc.dram_tensor("h0T", [128, H2], dt.bfloat16, kind="ExternalInput").ap()
    wbbx_d = nc.dram_tensor("wbbx", [128, 256], dt.bfloat16, kind="ExternalInput").ap()
    wbbh_d = nc.dram_tensor("wbbh", [128, 512], dt.bfloat16, kind="ExternalInput").ap()
    w4_d = nc.dram_tensor("w4", [128, 1536], dt.bfloat16, kind="ExternalInput").ap()
    bz_d = nc.dram_tensor("bias_z", [2, 128], dt.bfloat16, kind="ExternalInput").ap()
    bff_d = nc.dram_tensor("bias_ff", [4, 128], dt.bfloat16, kind="ExternalInput").ap()
    bd_d = nc.dram_tensor("bias_d", [2, 128], dt.bfloat16, kind="ExternalInput").ap()
    s2_d = nc.dram_tensor("sel2", [2, H2], dt.bfloat16, kind="ExternalInput").ap()
    s4_d = nc.dram_tensor("sel4", [4, 2 * H2], dt.bfloat16, kind="ExternalInput").ap()
    out_d = nc.dram_tensor("out", [128, seq_len, H2], dt.bfloat16,
                           kind="ExternalOutput").ap()

    n_chunks = seq_len // chunk
    assert seq_len % chunk == 0 and chunk % out_piece == 0

    with tile.TileContext(nc) as tc:
        _kernel_body(tc, xT, h0T, wbbx_d, wbbh_d, w4_d, bz_d, bff_d, bd_d,
                     s2_d, s4_d, out_d, seq_len, chunk, out_piece, n_chunks)
    nc.compile()
    return nc


@with_exitstack
def _kernel_body(ctx, tc, xT, h0T, wbbx_d, wbbh_d, w4_d, bz_d, bff_d, bd_d,
                 s2_d, s4_d, out_d, seq_len, chunk, out_piece, n_chunks):
    nc = tc.nc
    dt = mybir.dt
    f32 = dt.float32
    bf16 = dt.bfloat16

    wpool = ctx.enter_context(tc.tile_pool(name="weights", bufs=1))
    xpool = ctx.enter_context(tc.tile_pool(name="xin", bufs=2))
    hpool = ctx.enter_context(tc.tile_pool(name="hring", bufs=2))
    psum = ctx.enter_context(tc.tile_pool(name="psum", bufs=2, space="PSUM"))
    apool = ctx.enter_context(tc.tile_pool(name="acts", bufs=3))

    # resident weights / constants
    w_x = wpool.tile([128, 256], bf16, tag="w_x")
    w_h = wpool.tile([128, 512], bf16, tag="w_h")
    w_4 = wpool.tile([128, 1536], bf16, tag="w_4")
    b_z = wpool.tile([2, 128], bf16, tag="b_z")
    b_ff = wpool.tile([4, 128], bf16, tag="b_ff")
    b_d = wpool.tile([2, 128], bf16, tag="b_d")
    s2 = wpool.tile([2, H2], bf16, tag="s2")
    s4 = wpool.tile([4, 2 * H2], bf16, tag="s4")
    h_state = wpool.tile([128, H2], bf16, tag="h_state")

    nc.sync.dma_start(w_x[:], wbbx_d[:])
    nc.sync.dma_start(w_h[:], wbbh_d[:])
    nc.sync.dma_start(w_4[:], w4_d[:])
    nc.sync.dma_start(b_z[:], bz_d[:])
    nc.sync.dma_start(b_ff[:], bff_d[:])
    nc.sync.dma_start(b_d[:], bd_d[:])
    nc.sync.dma_start(s2[:], s2_d[:])
    nc.sync.dma_start(s4[:], s4_d[:])
    nc.sync.dma_start(h_state[:], h0T[:])

    for c in range(n_chunks):
        t0 = c * chunk
        x_t = xpool.tile([128, chunk * BC], bf16, tag="x_t")
        # xT[:, t0:t0+chunk, :] is (128, chunk, BC); contiguous per partition
        nc.sync.dma_start(x_t[:], xT[:, t0:t0 + chunk, :].rearrange("p t b -> p (t b)"))
        h_r = hpool.tile([128, chunk * H2], bf16, tag="h_r")

        for j in range(chunk):
            h_prev = h_state if j == 0 else h_r[:, (j - 1) * H2: j * H2]

            # ---- mm1: zpre = bbb + Wbbx.T@x_t + Wbbh.T@h  (feature-major) ----
            zp = psum.tile([128, H2], f32, tag="zpre")
            nc.tensor.matmul(zp[:, 0:H2], b_z[:], s2[:], start=True, stop=False)
            xr = x_t[:, j * BC:(j + 1) * BC]
            nc.tensor.matmul(zp[:, 0:BC], w_x[:, 0:128], xr, start=False, stop=False)
            nc.tensor.matmul(zp[:, BC:H2], w_x[:, 128:256], xr, start=False, stop=False)
            for k in (0, 1):
                hr = h_prev[:, k * BC:(k + 1) * BC]
                last = k == 1
                nc.tensor.matmul(zp[:, 0:BC], w_h[:, k * 256: k * 256 + 128],
                                 hr, start=False, stop=False)
                nc.tensor.matmul(zp[:, BC:H2], w_h[:, k * 256 + 128: k * 256 + 256],
                                 hr, start=False, stop=last)

            z = apool.tile([128, H2], bf16, tag="z")
            nc.scalar.activation(z[:], zp[:], AF.Tanh, scale=0.666)

            # ---- mm2: delta first (sigmoid overlaps ff matmuls) ----
            dl = psum.tile([128, H2], f32, tag="delta")
            nc.tensor.matmul(dl[:, 0:H2], b_d[:], s2[:], start=True, stop=False)
            ffp = psum.tile([128, 2 * H2], f32, tag="ff")
            nc.tensor.matmul(ffp[:, 0:2 * H2], b_ff[:], s4[:], start=True, stop=False)
            for k in (0, 1):
                zk = z[:, k * BC:(k + 1) * BC]
                ko = k * 768
                last = k == 1
                nc.tensor.matmul(dl[:, 0:BC], w_4[:, ko: ko + 128], zk,
                                 start=False, stop=False)
                nc.tensor.matmul(dl[:, BC:H2], w_4[:, ko + 128: ko + 256], zk,
                                 start=False, stop=last)
            sg = apool.tile([128, H2], f32, tag="s")
            nc.scalar.activation(sg[:], dl[:], AF.Sigmoid)

            for k in (0, 1):
                zk = z[:, k * BC:(k + 1) * BC]
                ko = k * 768
                last = k == 1
                nc.tensor.matmul(ffp[:, 0:BC], w_4[:, ko + 256: ko + 384], zk,
                                 start=False, stop=False)
                nc.tensor.matmul(ffp[:, BC:H2], w_4[:, ko + 384: ko + 512], zk,
                                 start=False, stop=False)
                nc.tensor.matmul(ffp[:, H2:H2 + BC], w_4[:, ko + 512: ko + 640], zk,
                                 start=False, stop=False)
                nc.tensor.matmul(ffp[:, H2 + BC:2 * H2], w_4[:, ko + 640: ko + 768],
                                 zk, start=False, stop=last)
            ffs = apool.tile([128, 2 * H2], f32, tag="ffs")
            nc.scalar.activation(ffs[:], ffp[:], AF.Tanh)

            # ---- gating (fp32), h out in bf16 ----
            d = apool.tile([128, H2], f32, tag="d")
            nc.vector.tensor_sub(d[:], ffs[:, H2:2 * H2], ffs[:, 0:H2])
            sd = apool.tile([128, H2], f32, tag="sd")
            nc.vector.tensor_mul(sd[:], sg[:], d[:])
            nc.vector.tensor_add(h_r[:, j * H2:(j + 1) * H2], ffs[:, 0:H2], sd[:])

            if (j + 1) % out_piece == 0:
                p0 = j + 1 - out_piece
                nc.sync.dma_start(
                    out_d[:, t0 + p0: t0 + j + 1, :].rearrange("p t b -> p (t b)"),
                    h_r[:, p0 * H2:(j + 1) * H2])

        nc.vector.tensor_copy(h_state[:], h_r[:, (chunk - 1) * H2: chunk * H2])


# ---------------------------------------------------------------------------
# host side
# ---------------------------------------------------------------------------

def prepare_in_maps(x, h0, Wbb, bbb, Wff1, bff1, Wff2, bff2, Wta, bta, Wtb, btb,
                    seq_len=L):
    f32 = np.float32
    Wbb = np.asarray(Wbb, f32)
    W4 = 1.7159 * np.concatenate(
        [np.asarray(Wtb, f32) - np.asarray(Wta, f32),
         np.asarray(Wff1, f32), np.asarray(Wff2, f32)], axis=1)  # (256, 768)
    # w4 sbuf layout: [k0 | k1] along cols
    w4_host = np.concatenate([W4[0:128, :], W4[128:256, :]], axis=1).astype(BF16)
    wbbx_host = Wbb[0:128, :].astype(BF16)                      # (128, 256)
    wbbh_host = np.concatenate([Wbb[128:256, :], Wbb[256:384, :]], axis=1).astype(BF16)

    bbb = np.asarray(bbb, f32); bff1 = np.asarray(bff1, f32)
    bff2 = np.asarray(bff2, f32)
    db = np.asarray(btb, f32) - np.asarray(bta, f32)
    bz_host = np.stack([bbb[0:128], bbb[128:256]]).astype(BF16)          # (2,128)
    bff_host = np.stack([bff1[0:128], bff1[128:256],
                         bff2[0:128], bff2[128:256]]).astype(BF16)       # (4,128)
    bd_host = np.stack([db[0:128], db[128:256]]).astype(BF16)            # (2,128)

    s2_host = np.zeros((2, H2), f32)
    s2_host[0, 0:BC] = 1.0
    s2_host[1, BC:H2] = 1.0
    s2_host = s2_host.astype(BF16)
    s4_host = np.zeros((4, 2 * H2), f32)
    for g in range(4):
        s4_host[g, g * BC:(g + 1) * BC] = 1.0
    s4_host = s4_host.astype(BF16)

    x = np.asarray(x, f32)
    h0 = np.asarray(h0, f32)
    in_maps = []
    for i in range(N_CORES):
        xs = x[i * BC:(i + 1) * BC, :seq_len, :]                # (BC, seq, C)
        xT_host = np.ascontiguousarray(xs.transpose(2, 1, 0)).astype(BF16)
        h0s = h0[i * BC:(i + 1) * BC]                           # (BC, U)
        h0T_host = np.concatenate(
            [h0s[:, 0:128].T, h0s[:, 128:256].T], axis=1).astype(BF16)  # (128, 64)
        in_maps.append({
            "xT": xT_host, "h0T": h0T_host,
            "wbbx": wbbx_host, "wbbh": wbbh_host, "w4": w4_host,
            "bias_z": bz_host, "bias_ff": bff_host, "bias_d": bd_host,
            "sel2": s2_host, "sel4": s4_host,
        })
    return in_maps


def assemble_output(results, seq_len=L):
    """results: list of {'out': (128, seq, 64) bf16} per core -> (B, seq, U) f32."""
    full = np.empty((B, seq_len, U), np.float32)
    for i, r in enumerate(results):
        o = np.asarray(r["out"]).astype(np.float32)      # (128, seq, 64)
        o = o.reshape(128, seq_len, 2, BC)               # p, t, g, b
        full[i * BC:(i + 1) * BC] = o.transpose(3, 1, 2, 0).reshape(BC, seq_len, U)
    return full


_NC_CACHE = {}


def _get_nc(seq_len=L, chunk=256):
    key = (seq_len, chunk)
    if key not in _NC_CACHE:
        _NC_CACHE[key] = build_nc(seq_len=seq_len, chunk=chunk)
    return _NC_CACHE[key]


def kernel(x, h0, Wbb, bbb, Wff1, bff1, Wff2, bff2, Wta, bta, Wtb, btb,
           _trace=False, _seq_len=L, _chunk=256):
    nc = _get_nc(_seq_len, _chunk)
    in_maps = prepare_in_maps(x, h0, Wbb, bbb, Wff1, bff1, Wff2, bff2,
                              Wta, bta, Wtb, btb, seq_len=_seq_len)
    res = bass_utils.run_bass_kernel_spmd(
        nc, in_maps, core_ids=list(range(N_CORES)), trace=_trace)
    readout = assemble_output(res.results, seq_len=_seq_len)
    hN = readout[:, -1, :].copy()
    kernel._last_results = res
    return readout, hN


# revision 9
# speedup vs baseline: 1.2211x; 1.2211x over previous
"""CfC (closed-form continuous-time) RNN kernel for Trainium2, 8 NeuronCores.

Problem: B=256, L=1024, C=128, U=BBU=256.
    step: z = 1.7159*tanh(0.666*(concat(x_t,h) @ Wbb + bbb))
          ff1 = tanh(z@Wff1+bff1); ff2 = tanh(z@Wff2+bff2)
          s = sigmoid(z@(Wtb-Wta) + (btb-bta))
          h = ff1 + s*(ff2-ff1)     (emitted every step)

Strategy: data-parallel over batch (32 per core). Everything feature-major:
features on SBUF partitions, batch on the free dim. Weights stay SBUF-resident
as bf16 128x128 stationary tiles (FWL); activations stream as the bf16 moving
operand (N=32). Biases are injected into PSUM with tiny K=2/K=4 matmuls against
constant selector matrices so the ACT/DVE ops need no per-slice bias handling.
Gating math runs fp32 on ACT/DVE; h is rounded to bf16 once per step (feeds
both the recurrence and the output DMA).
"""

import numpy as np
import ml_dtypes

import concourse.bass as bass
import concourse.mybir as mybir
import concourse.tile as tile
from concourse import bacc, bass_utils
from concourse._compat import with_exitstack

BF16 = ml_dtypes.bfloat16
AF = mybir.ActivationFunctionType

B, L, C, U, BBU = 256, 1024, 128, 256, 256
N_CORES = 8
BC = B // N_CORES  # 32 batch per core
H2 = 2 * BC        # 64: [u0|u1] concatenated along free dim


def build_nc(seq_len=L, chunk=256, out_piece=32):
    """Build the Bass/Tile program (same NEFF for all 8 cores)."""
    out_piece = min(out_piece, chunk)
    nc = bacc.Bacc("TRN2", target_bir_lowering=False, debug=False,
                   num_devices=N_CORES)
    dt = mybir.dt

    xT = nc.dram_tensor("xT", [C, seq_len, BC], dt.bfloat16, kind="ExternalInput").ap()
    h0T = nc.dram_tensor("h0T", [128, H2], dt.bfloat16, kind="ExternalInput").ap()
    wbbx_d = nc.dram_tensor("wbbx", [128, 256], dt.bfloat16, kind="ExternalInput").ap()
    wbbh_d = nc.dram_tensor("wbbh", [128, 512], dt.bfloat16, kind="ExternalInput").ap()
    w4_d = nc.dram_tensor("w4", [128, 1536], dt.bfloat16, kind="ExternalInput").ap()
    bz_d = nc.dram_tensor("bias_z", [2, 128], dt.bfloat16, kind="ExternalInput").ap()
    bff_d = nc.dram_tensor("bias_ff", [4, 128], dt.bfloat16, kind="ExternalInput").ap()
    bd_d = nc.dram_tensor("bias_d", [2, 128], dt.bfloat16, kind="ExternalInput").ap()
    s2_d = nc.dram_tensor("sel2", [2, H2], dt.bfloat16, kind="ExternalInput").ap()
    s4_d = nc.dram_tensor("sel4", [4, 2 * H2], dt.bfloat16, kind="ExternalInput").ap()
    out_d = nc.dram_tensor("out", [128, seq_len, H2], dt.bfloat16,
                           kind="ExternalOutput").ap()

    n_chunks = seq_len // chunk
    assert seq_len % chunk == 0 and chunk % out_piece == 0

    with tile.TileContext(nc) as tc:
        _kernel_body(tc, xT, h0T, wbbx_d, wbbh_d, w4_d, bz_d, bff_d, bd_d,
                     s2_d, s4_d, out_d, seq_len, chunk, out_piece, n_chunks)
    nc.compile()
    return nc


@with_exitstack
def _kernel_body(ctx, tc, xT, h0T, wbbx_d, wbbh_d, w4_d, bz_d, bff_d, bd_d,
                 s2_d, s4_d, out_d, seq_len, chunk, out_piece, n_chunks):
    nc = tc.nc
    dt = mybir.dt
    f32 = dt.float32
    bf16 = dt.bfloat16

    wpool = ctx.enter_context(tc.tile_pool(name="weights", bufs=1))
    xpool = ctx.enter_context(tc.tile_pool(name="xin", bufs=2))
    hpool = ctx.enter_context(tc.tile_pool(name="hring", bufs=2))
    psum = ctx.enter_context(tc.tile_pool(name="psum", bufs=2, space="PSUM"))
    apool = ctx.enter_context(tc.tile_pool(name="acts", bufs=3))

    # resident weights / constants
    w_x = wpool.tile([128, 256], bf16, tag="w_x")
    w_h = wpool.tile([128, 512], bf16, tag="w_h")
    w_4 = wpool.tile([128, 1536], bf16, tag="w_4")
    b_z = wpool.tile([2, 128], bf16, tag="b_z")
    b_ff = wpool.tile([4, 128], bf16, tag="b_ff")
    b_d = wpool.tile([2, 128], bf16, tag="b_d")
    s2 = wpool.tile([2, H2], bf16, tag="s2")
    s4 = wpool.tile([4, 2 * H2], bf16, tag="s4")
    h_state = wpool.tile([128, H2], bf16, tag="h_state")

    nc.sync.dma_start(w_x[:], wbbx_d[:])
    nc.sync.dma_start(w_h[:], wbbh_d[:])
    nc.sync.dma_start(w_4[:], w4_d[:])
    nc.sync.dma_start(b_z[:], bz_d[:])
    nc.sync.dma_start(b_ff[:], bff_d[:])
    nc.sync.dma_start(b_d[:], bd_d[:])
    nc.sync.dma_start(s2[:], s2_d[:])
    nc.sync.dma_start(s4[:], s4_d[:])
    nc.sync.dma_start(h_state[:], h0T[:])

    for c in range(n_chunks):
        t0 = c * chunk
        x_t = xpool.tile([128, chunk * BC], bf16, tag="x_t")
        # xT[:, t0:t0+chunk, :] is (128, chunk, BC); contiguous per partition
        nc.sync.dma_start(x_t[:], xT[:, t0:t0 + chunk, :].rearrange("p t b -> p (t b)"))
        h_r = hpool.tile([128, chunk * H2], bf16, tag="h_r")

        for j in range(chunk):
            h_prev = h_state if j == 0 else h_r[:, (j - 1) * H2: j * H2]

            # ---- mm1: zpre = bbb + Wbbx.T@x_t + Wbbh.T@h  (feature-major) ----
            zp = psum.tile([128, H2], f32, tag="zpre")
            nc.tensor.matmul(zp[:, 0:H2], b_z[:], s2[:], start=True, stop=False)
            xr = x_t[:, j * BC:(j + 1) * BC]
            nc.tensor.matmul(zp[:, 0:BC], w_x[:, 0:128], xr, start=False, stop=False)
            nc.tensor.matmul(zp[:, BC:H2], w_x[:, 128:256], xr, start=False, stop=False)
            for k in (0, 1):
                hr = h_prev[:, k * BC:(k + 1) * BC]
                last = k == 1
                nc.tensor.matmul(zp[:, 0:BC], w_h[:, k * 256: k * 256 + 128],
                                 hr, start=False, stop=False)
                nc.tensor.matmul(zp[:, BC:H2], w_h[:, k * 256 + 128: k * 256 + 256],
                                 hr, start=False, stop=last)

            z = apool.tile([128, H2], bf16, tag="z")
            nc.scalar.activation(z[:], zp[:], AF.Tanh, scale=0.666)

            # ---- mm2: delta first (sigmoid overlaps ff matmuls) ----
            dl = psum.tile([128, H2], f32, tag="delta")
            nc.tensor.matmul(dl[:, 0:H2], b_d[:], s2[:], start=True, stop=False)
            # cols 0:128 accumulate the ff matmuls; cols 128:256 hold the
            # interleaved tanh output (same bank, ACT-written only)
            ffp = psum.tile([128, 4 * H2], f32, tag="ff")
            nc.tensor.matmul(ffp[:, 0:2 * H2], b_ff[:], s4[:], start=True, stop=False)
            for k in (0, 1):
                zk = z[:, k * BC:(k + 1) * BC]
                ko = k * 768
                last = k == 1
                nc.tensor.matmul(dl[:, 0:BC], w_4[:, ko: ko + 128], zk,
                                 start=False, stop=False)
                nc.tensor.matmul(dl[:, BC:H2], w_4[:, ko + 128: ko + 256], zk,
                                 start=False, stop=last)
            # s_i holds interleaved [1-s | s] pairs: even slots sigmoid(-delta),
            # odd slots sigmoid(delta). Both ACT ops run while ff matmuls stream.
            sg = apool.tile([128, 2 * H2], f32, tag="s")
            sg3 = sg[:].rearrange("p (a b) -> p b a", b=2)
            nc.scalar.activation(sg3[:, 0, :], dl[:], AF.Sigmoid, scale=-1.0)
            nc.scalar.activation(sg3[:, 1, :], dl[:], AF.Sigmoid)

            for k in (0, 1):
                zk = z[:, k * BC:(k + 1) * BC]
                ko = k * 768
                last = k == 1
                nc.tensor.matmul(ffp[:, 0:BC], w_4[:, ko + 256: ko + 384], zk,
                                 start=False, stop=False)
                nc.tensor.matmul(ffp[:, BC:H2], w_4[:, ko + 384: ko + 512], zk,
                                 start=False, stop=False)
                nc.tensor.matmul(ffp[:, H2:H2 + BC], w_4[:, ko + 512: ko + 640], zk,
                                 start=False, stop=False)
                nc.tensor.matmul(ffp[:, H2 + BC:2 * H2], w_4[:, ko + 640: ko + 768],
                                 zk, start=False, stop=last)
            # tanh(ff_psum) -> interleaved [ff1|ff2] pairs, written back into the
            # same PSUM bank (cols 128:256) — ScalarE's fast port is PSUM.
            ff_in3 = ffp[:, 0:2 * H2].rearrange("p (b a) -> p b a", b=2)
            ff_out3 = ffp[:, 2 * H2:4 * H2].rearrange("p (a b) -> p b a", b=2)
            nc.scalar.activation(ff_out3[:], ff_in3[:], AF.Tanh)

            # ---- gating: h = (1-s)*ff1 + s*ff2, fp32, h out in bf16 ----
            prod = apool.tile([128, 2 * H2], f32, tag="prod")
            nc.vector.tensor_mul(prod[:], ffp[:, 2 * H2:4 * H2], sg[:])
            prod3 = prod[:].rearrange("p (a b) -> p b a", b=2)
            nc.vector.tensor_add(h_r[:, j * H2:(j + 1) * H2],
                                 prod3[:, 0, :], prod3[:, 1, :])

            if (j + 1) % out_piece == 0:
                p0 = j + 1 - out_piece
                nc.sync.dma_start(
                    out_d[:, t0 + p0: t0 + j + 1, :].rearrange("p t b -> p (t b)"),
                    h_r[:, p0 * H2:(j + 1) * H2])

        nc.vector.tensor_copy(h_state[:], h_r[:, (chunk - 1) * H2: chunk * H2])


# ---------------------------------------------------------------------------
# host side
# ---------------------------------------------------------------------------

def prepare_in_maps(x, h0, Wbb, bbb, Wff1, bff1, Wff2, bff2, Wta, bta, Wtb, btb,
                    seq_len=L):
    f32 = np.float32
    Wbb = np.asarray(Wbb, f32)
    W4 = 1.7159 * np.concatenate(
        [np.asarray(Wtb, f32) - np.asarray(Wta, f32),
         np.asarray(Wff1, f32), np.asarray(Wff2, f32)], axis=1)  # (256, 768)
    # w4 sbuf layout: [k0 | k1] along cols
    w4_host = np.concatenate([W4[0:128, :], W4[128:256, :]], axis=1).astype(BF16)
    wbbx_host = Wbb[0:128, :].astype(BF16)                      # (128, 256)
    wbbh_host = np.concatenate([Wbb[128:256, :], Wbb[256:384, :]], axis=1).astype(BF16)

    bbb = np.asarray(bbb, f32); bff1 = np.asarray(bff1, f32)
    bff2 = np.asarray(bff2, f32)
    db = np.asarray(btb, f32) - np.asarray(bta, f32)
    bz_host = np.stack([bbb[0:128], bbb[128:256]]).astype(BF16)          # (2,128)
    bff_host = np.stack([bff1[0:128], bff1[128:256],
                         bff2[0:128], bff2[128:256]]).astype(BF16)       # (4,128)
    bd_host = np.stack([db[0:128], db[128:256]]).astype(BF16)            # (2,128)

    s2_host = np.zeros((2, H2), f32)
    s2_host[0, 0:BC] = 1.0
    s2_host[1, BC:H2] = 1.0
    s2_host = s2_host.astype(BF16)
    s4_host = np.zeros((4, 2 * H2), f32)
    for g in range(4):
        s4_host[g, g * BC:(g + 1) * BC] = 1.0
    s4_host = s4_host.astype(BF16)

    x = np.asarray(x, f32)
    h0 = np.asarray(h0, f32)
    in_maps = []
    for i in range(N_CORES):
        xs = x[i * BC:(i + 1) * BC, :seq_len, :]                # (BC, seq, C)
        xT_host = np.ascontiguousarray(xs.transpose(2, 1, 0)).astype(BF16)
        h0s = h0[i * BC:(i + 1) * BC]                           # (BC, U)
        h0T_host = np.concatenate(
            [h0s[:, 0:128].T, h0s[:, 128:256].T], axis=1).astype(BF16)  # (128, 64)
        in_maps.append({
            "xT": xT_host, "h0T": h0T_host,
            "wbbx": wbbx_host, "wbbh": wbbh_host, "w4": w4_host,
            "bias_z": bz_host, "bias_ff": bff_host, "bias_d": bd_host,
            "sel2": s2_host, "sel4": s4_host,
        })
    return in_maps


def assemble_output(results, seq_len=L):
    """results: list of {'out': (128, seq, 64) bf16} per core -> (B, seq, U) f32."""
    full = np.empty((B, seq_len, U), np.float32)
    for i, r in enumerate(results):
        o = np.asarray(r["out"]).astype(np.float32)      # (128, seq, 64)
        o = o.reshape(128, seq_len, 2, BC)               # p, t, g, b
        full[i * BC:(i + 1) * BC] = o.transpose(3, 1, 2, 0).reshape(BC, seq_len, U)
    return full


_NC_CACHE = {}


def _get_nc(seq_len=L, chunk=256):
    key = (seq_len, chunk)
    if key not in _NC_CACHE:
        _NC_CACHE[key] = build_nc(seq_len=seq_len, chunk=chunk)
    return _NC_CACHE[key]


def kernel(x, h0, Wbb, bbb, Wff1, bff1, Wff2, bff2, Wta, bta, Wtb, btb,
           _trace=False, _seq_len=L, _chunk=256):
    nc = _get_nc(_seq_len, _chunk)
    in_maps = prepare_in_maps(x, h0, Wbb, bbb, Wff1, bff1, Wff2, bff2,
                              Wta, bta, Wtb, btb, seq_len=_seq_len)
    res = bass_utils.run_bass_kernel_spmd(
        nc, in_maps, core_ids=list(range(N_CORES)), trace=_trace)
    readout = assemble_output(res.results, seq_len=_seq_len)
    hN = readout[:, -1, :].copy()
    kernel._last_results = res
    return readout, hN


# revision 12
# speedup vs baseline: 1.2832x; 1.0508x over previous
"""CfC (closed-form continuous-time) RNN kernel for Trainium2, 8 NeuronCores.

Problem: B=256, L=1024, C=128, U=BBU=256.
    step: z = 1.7159*tanh(0.666*(concat(x_t,h) @ Wbb + bbb))
          ff1 = tanh(z@Wff1+bff1); ff2 = tanh(z@Wff2+bff2)
          s = sigmoid(z@(Wtb-Wta) + (btb-bta))
          h = ff1 + s*(ff2-ff1)     (emitted every step)

Strategy: data-parallel over batch (32 per core). Everything feature-major:
features on SBUF partitions, batch on the free dim. Weights stay SBUF-resident
as bf16 128x128 stationary tiles (FWL); activations stream as the bf16 moving
operand (N=32). Biases are injected into PSUM with tiny K=2/K=4 matmuls against
constant selector matrices so the ACT/DVE ops need no per-slice bias handling.
Gating math runs fp32 on ACT/DVE; h is rounded to bf16 once per step (feeds
both the recurrence and the output DMA).
"""

import numpy as np
import ml_dtypes

import concourse.bass as bass
import concourse.mybir as mybir
import concourse.tile as tile
from concourse import bacc, bass_utils
from concourse._compat import with_exitstack
from concourse.alu_op_type import AluOpType as ALU

BF16 = ml_dtypes.bfloat16
AF = mybir.ActivationFunctionType

B, L, C, U, BBU = 256, 1024, 128, 256, 256
N_CORES = 8
BC = B // N_CORES  # 32 batch per core
H2 = 2 * BC        # 64: [u0|u1] concatenated along free dim


def build_nc(seq_len=L, chunk=256, out_piece=32):
    """Build the Bass/Tile program (same NEFF for all 8 cores)."""
    out_piece = min(out_piece, chunk)
    nc = bacc.Bacc("TRN2", target_bir_lowering=False, debug=False,
                   num_devices=N_CORES)
    dt = mybir.dt

    xT = nc.dram_tensor("xT", [C, seq_len, BC], dt.bfloat16, kind="ExternalInput").ap()
    h0T = nc.dram_tensor("h0T", [128, H2], dt.bfloat16, kind="ExternalInput").ap()
    wbbx_d = nc.dram_tensor("wbbx", [128, 256], dt.bfloat16, kind="ExternalInput").ap()
    wbbh_d = nc.dram_tensor("wbbh", [128, 512], dt.bfloat16, kind="ExternalInput").ap()
    w4_d = nc.dram_tensor("w4", [128, 1536], dt.bfloat16, kind="ExternalInput").ap()
    bz_d = nc.dram_tensor("bias_z", [2, 128], dt.bfloat16, kind="ExternalInput").ap()
    bff_d = nc.dram_tensor("bias_ff", [4, 128], dt.bfloat16, kind="ExternalInput").ap()
    bd_d = nc.dram_tensor("bias_d", [2, 128], dt.bfloat16, kind="ExternalInput").ap()
    s2_d = nc.dram_tensor("sel2", [2, H2], dt.bfloat16, kind="ExternalInput").ap()
    s4_d = nc.dram_tensor("sel4", [4, 2 * H2], dt.bfloat16, kind="ExternalInput").ap()
    out_d = nc.dram_tensor("out", [128, seq_len, H2], dt.bfloat16,
                           kind="ExternalOutput").ap()

    n_chunks = seq_len // chunk
    assert seq_len % chunk == 0 and chunk % out_piece == 0

    with tile.TileContext(nc) as tc:
        _kernel_body(tc, xT, h0T, wbbx_d, wbbh_d, w4_d, bz_d, bff_d, bd_d,
                     s2_d, s4_d, out_d, seq_len, chunk, out_piece, n_chunks)
    nc.compile()
    return nc


@with_exitstack
def _kernel_body(ctx, tc, xT, h0T, wbbx_d, wbbh_d, w4_d, bz_d, bff_d, bd_d,
                 s2_d, s4_d, out_d, seq_len, chunk, out_piece, n_chunks):
    nc = tc.nc
    dt = mybir.dt
    f32 = dt.float32
    bf16 = dt.bfloat16

    wpool = ctx.enter_context(tc.tile_pool(name="weights", bufs=1))
    xpool = ctx.enter_context(tc.tile_pool(name="xin", bufs=2))
    hpool = ctx.enter_context(tc.tile_pool(name="hring", bufs=2))
    psum = ctx.enter_context(tc.tile_pool(name="psum", bufs=2, space="PSUM"))
    apool = ctx.enter_context(tc.tile_pool(name="acts", bufs=3))

    # resident weights / constants
    w_x = wpool.tile([128, 256], bf16, tag="w_x")
    w_h = wpool.tile([128, 512], bf16, tag="w_h")
    w_4 = wpool.tile([128, 1536], bf16, tag="w_4")
    b_z = wpool.tile([2, 128], bf16, tag="b_z")
    b_ff = wpool.tile([4, 128], bf16, tag="b_ff")
    b_d = wpool.tile([2, 128], bf16, tag="b_d")
    s2 = wpool.tile([2, H2], bf16, tag="s2")
    s4 = wpool.tile([4, 2 * H2], bf16, tag="s4")
    h_state = wpool.tile([128, H2], bf16, tag="h_state")

    nc.sync.dma_start(w_x[:], wbbx_d[:])
    nc.sync.dma_start(w_h[:], wbbh_d[:])
    nc.sync.dma_start(w_4[:], w4_d[:])
    nc.sync.dma_start(b_z[:], bz_d[:])
    nc.sync.dma_start(b_ff[:], bff_d[:])
    nc.sync.dma_start(b_d[:], bd_d[:])
    nc.sync.dma_start(s2[:], s2_d[:])
    nc.sync.dma_start(s4[:], s4_d[:])
    nc.sync.dma_start(h_state[:], h0T[:])

    for c in range(n_chunks):
        t0 = c * chunk
        x_t = xpool.tile([128, chunk * BC], bf16, tag="x_t")
        # xT[:, t0:t0+chunk, :] is (128, chunk, BC); contiguous per partition
        nc.sync.dma_start(x_t[:], xT[:, t0:t0 + chunk, :].rearrange("p t b -> p (t b)"))
        h_r = hpool.tile([128, chunk * H2], bf16, tag="h_r")

        for j in range(chunk):
            h_prev = h_state if j == 0 else h_r[:, (j - 1) * H2: j * H2]

            # ---- mm1: zpre = bbb + Wbbx.T@x_t + Wbbh.T@h  (feature-major) ----
            zp = psum.tile([128, H2], f32, tag="zpre")
            nc.tensor.matmul(zp[:, 0:H2], b_z[:], s2[:], start=True, stop=False)
            xr = x_t[:, j * BC:(j + 1) * BC]
            nc.tensor.matmul(zp[:, 0:BC], w_x[:, 0:128], xr, start=False, stop=False)
            nc.tensor.matmul(zp[:, BC:H2], w_x[:, 128:256], xr, start=False, stop=False)
            for k in (0, 1):
                hr = h_prev[:, k * BC:(k + 1) * BC]
                last = k == 1
                nc.tensor.matmul(zp[:, 0:BC], w_h[:, k * 256: k * 256 + 128],
                                 hr, start=False, stop=False)
                nc.tensor.matmul(zp[:, BC:H2], w_h[:, k * 256 + 128: k * 256 + 256],
                                 hr, start=False, stop=last)

            z = apool.tile([128, H2], bf16, tag="z")
            nc.scalar.activation(z[:], zp[:], AF.Tanh, scale=0.666)

            # ---- mm2: delta first (sigmoid overlaps ff matmuls) ----
            dl = psum.tile([128, H2], f32, tag="delta")
            nc.tensor.matmul(dl[:, 0:H2], b_d[:], s2[:], start=True, stop=False)
            # cols 0:128 accumulate the ff matmuls; cols 128:256 hold the
            # interleaved tanh output (same bank, ACT-written only)
            ffp = psum.tile([128, 4 * H2], f32, tag="ff")
            nc.tensor.matmul(ffp[:, 0:2 * H2], b_ff[:], s4[:], start=True, stop=False)
            for k in (0, 1):
                zk = z[:, k * BC:(k + 1) * BC]
                ko = k * 768
                last = k == 1
                nc.tensor.matmul(dl[:, 0:BC], w_4[:, ko: ko + 128], zk,
                                 start=False, stop=False)
                nc.tensor.matmul(dl[:, BC:H2], w_4[:, ko + 128: ko + 256], zk,
                                 start=False, stop=last)
            # s_i holds interleaved [1-s | s] pairs: sigmoid on ACT (odd slots),
            # 1-s on DVE (even slots) — keeps ACT at 3 ops/step.
            sg = apool.tile([128, 2 * H2], f32, tag="s")
            sg3 = sg[:].rearrange("p (a b) -> p b a", b=2)
            nc.scalar.activation(sg3[:, 1, :], dl[:], AF.Sigmoid)
            nc.vector.tensor_scalar(sg3[:, 0, :], sg3[:, 1, :], -1.0, 1.0,
                                    ALU.mult, ALU.add)

            for k in (0, 1):
                zk = z[:, k * BC:(k + 1) * BC]
                ko = k * 768
                last = k == 1
                nc.tensor.matmul(ffp[:, 0:BC], w_4[:, ko + 256: ko + 384], zk,
                                 start=False, stop=False)
                nc.tensor.matmul(ffp[:, BC:H2], w_4[:, ko + 384: ko + 512], zk,
                                 start=False, stop=False)
                nc.tensor.matmul(ffp[:, H2:H2 + BC], w_4[:, ko + 512: ko + 640], zk,
                                 start=False, stop=False)
                nc.tensor.matmul(ffp[:, H2 + BC:2 * H2], w_4[:, ko + 640: ko + 768],
                                 zk, start=False, stop=last)
            # tanh(ff_psum) -> interleaved [ff1|ff2] pairs, written back into the
            # same PSUM bank (cols 128:256) — ScalarE's fast port is PSUM.
            ff_in3 = ffp[:, 0:2 * H2].rearrange("p (b a) -> p b a", b=2)
            ff_out3 = ffp[:, 2 * H2:4 * H2].rearrange("p (a b) -> p b a", b=2)
            nc.scalar.activation(ff_out3[:], ff_in3[:], AF.Tanh)

            # ---- gating: h = (1-s)*ff1 + s*ff2, fp32, h out in bf16 ----
            prod = apool.tile([128, 2 * H2], f32, tag="prod")
            nc.vector.tensor_mul(prod[:], ffp[:, 2 * H2:4 * H2], sg[:])
            prod3 = prod[:].rearrange("p (a b) -> p b a", b=2)
            nc.vector.tensor_add(h_r[:, j * H2:(j + 1) * H2],
                                 prod3[:, 0, :], prod3[:, 1, :])

            if (j + 1) % out_piece == 0:
                p0 = j + 1 - out_piece
                nc.sync.dma_start(
                    out_d[:, t0 + p0: t0 + j + 1, :].rearrange("p t b -> p (t b)"),
                    h_r[:, p0 * H2:(j + 1) * H2])

        nc.vector.tensor_copy(h_state[:], h_r[:, (chunk - 1) * H2: chunk * H2])


# ---------------------------------------------------------------------------
# host side
# ---------------------------------------------------------------------------

def prepare_in_maps(x, h0, Wbb, bbb, Wff1, bff1, Wff2, bff2, Wta, bta, Wtb, btb,
                    seq_len=L):
    f32 = np.float32
    Wbb = np.asarray(Wbb, f32)
    W4 = 1.7159 * np.concatenate(
        [np.asarray(Wtb, f32) - np.asarray(Wta, f32),
         np.asarray(Wff1, f32), np.asarray(Wff2, f32)], axis=1)  # (256, 768)
    # w4 sbuf layout: [k0 | k1] along cols
    w4_host = np.concatenate([W4[0:128, :], W4[128:256, :]], axis=1).astype(BF16)
    wbbx_host = Wbb[0:128, :].astype(BF16)                      # (128, 256)
    wbbh_host = np.concatenate([Wbb[128:256, :], Wbb[256:384, :]], axis=1).astype(BF16)

    bbb = np.asarray(bbb, f32); bff1 = np.asarray(bff1, f32)
    bff2 = np.asarray(bff2, f32)
    db = np.asarray(btb, f32) - np.asarray(bta, f32)
    bz_host = np.stack([bbb[0:128], bbb[128:256]]).astype(BF16)          # (2,128)
    bff_host = np.stack([bff1[0:128], bff1[128:256],
                         bff2[0:128], bff2[128:256]]).astype(BF16)       # (4,128)
    bd_host = np.stack([db[0:128], db[128:256]]).astype(BF16)            # (2,128)

    s2_host = np.zeros((2, H2), f32)
    s2_host[0, 0:BC] = 1.0
    s2_host[1, BC:H2] = 1.0
    s2_host = s2_host.astype(BF16)
    s4_host = np.zeros((4, 2 * H2), f32)
    for g in range(4):
        s4_host[g, g * BC:(g + 1) * BC] = 1.0
    s4_host = s4_host.astype(BF16)

    x = np.asarray(x, f32)
    h0 = np.asarray(h0, f32)
    in_maps = []
    for i in range(N_CORES):
        xs = x[i * BC:(i + 1) * BC, :seq_len, :]                # (BC, seq, C)
        xT_host = np.ascontiguousarray(xs.transpose(2, 1, 0)).astype(BF16)
        h0s = h0[i * BC:(i + 1) * BC]                           # (BC, U)
        h0T_host = np.concatenate(
            [h0s[:, 0:128].T, h0s[:, 128:256].T], axis=1).astype(BF16)  # (128, 64)
        in_maps.append({
            "xT": xT_host, "h0T": h0T_host,
            "wbbx": wbbx_host, "wbbh": wbbh_host, "w4": w4_host,
            "bias_z": bz_host, "bias_ff": bff_host, "bias_d": bd_host,
            "sel2": s2_host, "sel4": s4_host,
        })
    return in_maps


def assemble_output(results, seq_len=L):
    """results: list of {'out': (128, seq, 64) bf16} per core -> (B, seq, U) f32."""
    full = np.empty((B, seq_len, U), np.float32)
    for i, r in enumerate(results):
        o = np.asarray(r["out"]).astype(np.float32)      # (128, seq, 64)
        o = o.reshape(128, seq_len, 2, BC)               # p, t, g, b
        full[i * BC:(i + 1) * BC] = o.transpose(3, 1, 2, 0).reshape(BC, seq_len, U)
    return full


_NC_CACHE = {}


def _get_nc(seq_len=L, chunk=256):
    key = (seq_len, chunk)
    if key not in _NC_CACHE:
        _NC_CACHE[key] = build_nc(seq_len=seq_len, chunk=chunk)
    return _NC_CACHE[key]


def kernel(x, h0, Wbb, bbb, Wff1, bff1, Wff2, bff2, Wta, bta, Wtb, btb,
           _trace=False, _seq_len=L, _chunk=256):
    nc = _get_nc(_seq_len, _chunk)
    in_maps = prepare_in_maps(x, h0, Wbb, bbb, Wff1, bff1, Wff2, bff2,
                              Wta, bta, Wtb, btb, seq_len=_seq_len)
    res = bass_utils.run_bass_kernel_spmd(
        nc, in_maps, core_ids=list(range(N_CORES)), trace=_trace)
    readout = assemble_output(res.results, seq_len=_seq_len)
    hN = readout[:, -1, :].copy()
    kernel._last_results = res
    return readout, hN


# revision 14
# speedup vs baseline: 1.3823x; 1.0772x over previous
"""CfC (closed-form continuous-time) RNN kernel for Trainium2, 8 NeuronCores.

Problem: B=256, L=1024, C=128, U=BBU=256.
    step: z = 1.7159*tanh(0.666*(concat(x_t,h) @ Wbb + bbb))
          ff1 = tanh(z@Wff1+bff1); ff2 = tanh(z@Wff2+bff2)
          s = sigmoid(z@(Wtb-Wta) + (btb-bta))
          h = ff1 + s*(ff2-ff1)     (emitted every step)

Strategy: data-parallel over batch (32 per core). Everything feature-major:
features on SBUF partitions, batch on the free dim. Weights stay SBUF-resident
as bf16 128x128 stationary tiles (FWL); activations stream as the bf16 moving
operand (N=32). Biases are injected into PSUM with tiny K=2/K=4 matmuls against
constant selector matrices so the ACT/DVE ops need no per-slice bias handling.
Gating math runs fp32 on ACT/DVE; h is rounded to bf16 once per step (feeds
both the recurrence and the output DMA).
"""

import numpy as np
import ml_dtypes

import concourse.bass as bass
import concourse.mybir as mybir
import concourse.tile as tile
from concourse import bacc, bass_utils
from concourse._compat import with_exitstack
from concourse.alu_op_type import AluOpType as ALU

BF16 = ml_dtypes.bfloat16
AF = mybir.ActivationFunctionType

B, L, C, U, BBU = 256, 1024, 128, 256, 256
N_CORES = 8
BC = B // N_CORES  # 32 batch per core
H2 = 2 * BC        # 64: [u0|u1] concatenated along free dim


def build_nc(seq_len=L, chunk=256, out_piece=32):
    """Build the Bass/Tile program (same NEFF for all 8 cores)."""
    out_piece = min(out_piece, chunk)
    nc = bacc.Bacc("TRN2", target_bir_lowering=False, debug=False,
                   num_devices=N_CORES)
    dt = mybir.dt

    xT = nc.dram_tensor("xT", [C, seq_len, BC], dt.bfloat16, kind="ExternalInput").ap()
    h0T = nc.dram_tensor("h0T", [128, H2], dt.bfloat16, kind="ExternalInput").ap()
    wbbx_d = nc.dram_tensor("wbbx", [128, 256], dt.bfloat16, kind="ExternalInput").ap()
    wbbh_d = nc.dram_tensor("wbbh", [128, 512], dt.bfloat16, kind="ExternalInput").ap()
    w4_d = nc.dram_tensor("w4", [128, 1536], dt.bfloat16, kind="ExternalInput").ap()
    bz_d = nc.dram_tensor("bias_z", [2, 128], dt.bfloat16, kind="ExternalInput").ap()
    bff_d = nc.dram_tensor("bias_ff", [4, 128], dt.bfloat16, kind="ExternalInput").ap()
    bd_d = nc.dram_tensor("bias_d", [2, 128], dt.bfloat16, kind="ExternalInput").ap()
    s2_d = nc.dram_tensor("sel2", [2, H2], dt.bfloat16, kind="ExternalInput").ap()
    s4_d = nc.dram_tensor("sel4", [4, 2 * H2], dt.bfloat16, kind="ExternalInput").ap()
    out_d = nc.dram_tensor("out", [128, seq_len, H2], dt.bfloat16,
                           kind="ExternalOutput").ap()

    n_chunks = seq_len // chunk
    assert seq_len % chunk == 0 and chunk % out_piece == 0

    with tile.TileContext(nc) as tc:
        _kernel_body(tc, xT, h0T, wbbx_d, wbbh_d, w4_d, bz_d, bff_d, bd_d,
                     s2_d, s4_d, out_d, seq_len, chunk, out_piece, n_chunks)
    nc.compile()
    return nc


@with_exitstack
def _kernel_body(ctx, tc, xT, h0T, wbbx_d, wbbh_d, w4_d, bz_d, bff_d, bd_d,
                 s2_d, s4_d, out_d, seq_len, chunk, out_piece, n_chunks):
    nc = tc.nc
    dt = mybir.dt
    f32 = dt.float32
    bf16 = dt.bfloat16

    wpool = ctx.enter_context(tc.tile_pool(name="weights", bufs=1))
    xpool = ctx.enter_context(tc.tile_pool(name="xin", bufs=2))
    hpool = ctx.enter_context(tc.tile_pool(name="hring", bufs=2))
    psum = ctx.enter_context(tc.tile_pool(name="psum", bufs=2, space="PSUM"))
    apool = ctx.enter_context(tc.tile_pool(name="acts", bufs=3))

    # resident weights / constants
    w_x = wpool.tile([128, 256], bf16, tag="w_x")
    w_h = wpool.tile([128, 512], bf16, tag="w_h")
    w_4 = wpool.tile([128, 1536], bf16, tag="w_4")
    b_z = wpool.tile([2, 128], bf16, tag="b_z")
    b_ff = wpool.tile([4, 128], bf16, tag="b_ff")
    b_d = wpool.tile([2, 128], bf16, tag="b_d")
    s2 = wpool.tile([2, H2], bf16, tag="s2")
    s4 = wpool.tile([4, 2 * H2], bf16, tag="s4")
    h_state = wpool.tile([128, H2], bf16, tag="h_state")

    nc.sync.dma_start(w_x[:], wbbx_d[:])
    nc.sync.dma_start(w_h[:], wbbh_d[:])
    nc.sync.dma_start(w_4[:], w4_d[:])
    nc.sync.dma_start(b_z[:], bz_d[:])
    nc.sync.dma_start(b_ff[:], bff_d[:])
    nc.sync.dma_start(b_d[:], bd_d[:])
    nc.sync.dma_start(s2[:], s2_d[:])
    nc.sync.dma_start(s4[:], s4_d[:])
    nc.sync.dma_start(h_state[:], h0T[:])

    for c in range(n_chunks):
        t0 = c * chunk
        x_t = xpool.tile([128, chunk * BC], bf16, tag="x_t")
        # xT[:, t0:t0+chunk, :] is (128, chunk, BC); contiguous per partition
        nc.sync.dma_start(x_t[:], xT[:, t0:t0 + chunk, :].rearrange("p t b -> p (t b)"))
        h_r = hpool.tile([128, chunk * H2], bf16, tag="h_r")

        prod_prev = None
        for j in range(chunk):
            # ---- mm1: zpre = bbb + Wbbx.T@x_t + Wbbh.T@h  (feature-major).
            # h = prod_even + prod_odd is folded into the matmul: Wh.T@h =
            # Wh.T@prod_even + Wh.T@prod_odd, so mm1 reads the strided bf16
            # halves of prod directly and the explicit h add stays off-chain.
            zp = psum.tile([128, H2], f32, tag="zpre")
            nc.tensor.matmul(zp[:, 0:H2], b_z[:], s2[:], start=True, stop=False)
            xr = x_t[:, j * BC:(j + 1) * BC]
            nc.tensor.matmul(zp[:, 0:BC], w_x[:, 0:128], xr, start=False, stop=False)
            nc.tensor.matmul(zp[:, BC:H2], w_x[:, 128:256], xr, start=False, stop=False)
            if prod_prev is None:
                for k in (0, 1):
                    hr = h_state[:, k * BC:(k + 1) * BC]
                    last = k == 1
                    nc.tensor.matmul(zp[:, 0:BC], w_h[:, k * 256: k * 256 + 128],
                                     hr, start=False, stop=False)
                    nc.tensor.matmul(zp[:, BC:H2], w_h[:, k * 256 + 128: k * 256 + 256],
                                     hr, start=False, stop=last)
            else:
                pp3 = prod_prev[:].rearrange("p (n two) -> p two n", two=2)
                for k in (0, 1):
                    for half in (0, 1):
                        hr = pp3[:, half, k * BC:(k + 1) * BC]
                        last = k == 1 and half == 1
                        nc.tensor.matmul(zp[:, 0:BC],
                                         w_h[:, k * 256: k * 256 + 128],
                                         hr, start=False, stop=False)
                        nc.tensor.matmul(zp[:, BC:H2],
                                         w_h[:, k * 256 + 128: k * 256 + 256],
                                         hr, start=False, stop=last)

            z = apool.tile([128, H2], bf16, tag="z")
            nc.scalar.activation(z[:], zp[:], AF.Tanh, scale=0.666)

            # ---- mm2: delta first (sigmoid overlaps ff matmuls) ----
            dl = psum.tile([128, H2], f32, tag="delta")
            nc.tensor.matmul(dl[:, 0:H2], b_d[:], s2[:], start=True, stop=False)
            # cols 0:128 accumulate the ff matmuls; cols 128:256 hold the
            # interleaved tanh output (same bank, ACT-written only)
            ffp = psum.tile([128, 4 * H2], f32, tag="ff")
            nc.tensor.matmul(ffp[:, 0:2 * H2], b_ff[:], s4[:], start=True, stop=False)
            for k in (0, 1):
                zk = z[:, k * BC:(k + 1) * BC]
                ko = k * 768
                last = k == 1
                nc.tensor.matmul(dl[:, 0:BC], w_4[:, ko: ko + 128], zk,
                                 start=False, stop=False)
                nc.tensor.matmul(dl[:, BC:H2], w_4[:, ko + 128: ko + 256], zk,
                                 start=False, stop=last)
            # s_i holds interleaved [1-s | s] pairs: sigmoid on ACT (odd slots),
            # 1-s on DVE (even slots) — keeps ACT at 3 ops/step.
            sg = apool.tile([128, 2 * H2], f32, tag="s")
            sg3 = sg[:].rearrange("p (a b) -> p b a", b=2)
            nc.scalar.activation(sg3[:, 1, :], dl[:], AF.Sigmoid)
            nc.vector.tensor_scalar(sg3[:, 0, :], sg3[:, 1, :], -1.0, 1.0,
                                    ALU.mult, ALU.add)

            for k in (0, 1):
                zk = z[:, k * BC:(k + 1) * BC]
                ko = k * 768
                last = k == 1
                nc.tensor.matmul(ffp[:, 0:BC], w_4[:, ko + 256: ko + 384], zk,
                                 start=False, stop=False)
                nc.tensor.matmul(ffp[:, BC:H2], w_4[:, ko + 384: ko + 512], zk,
                                 start=False, stop=False)
                nc.tensor.matmul(ffp[:, H2:H2 + BC], w_4[:, ko + 512: ko + 640], zk,
                                 start=False, stop=False)
                nc.tensor.matmul(ffp[:, H2 + BC:2 * H2], w_4[:, ko + 640: ko + 768],
                                 zk, start=False, stop=last)
            # tanh(ff_psum) -> interleaved [ff1|ff2] pairs, written back into the
            # same PSUM bank (cols 128:256) — ScalarE's fast port is PSUM.
            ff_in3 = ffp[:, 0:2 * H2].rearrange("p (b a) -> p b a", b=2)
            ff_out3 = ffp[:, 2 * H2:4 * H2].rearrange("p (a b) -> p b a", b=2)
            nc.scalar.activation(ff_out3[:], ff_in3[:], AF.Tanh)

            # ---- gating: h = (1-s)*ff1 + s*ff2 ----
            # prod in bf16 feeds the next step's matmuls directly; the explicit
            # pairwise add only serves the output ring (off the critical path).
            prod = apool.tile([128, 2 * H2], bf16, tag="prod")
            nc.vector.tensor_mul(prod[:], ffp[:, 2 * H2:4 * H2], sg[:])
            prod3 = prod[:].rearrange("p (a b) -> p b a", b=2)
            nc.vector.tensor_add(h_r[:, j * H2:(j + 1) * H2],
                                 prod3[:, 0, :], prod3[:, 1, :])
            prod_prev = prod

            if (j + 1) % out_piece == 0:
                p0 = j + 1 - out_piece
                nc.sync.dma_start(
                    out_d[:, t0 + p0: t0 + j + 1, :].rearrange("p t b -> p (t b)"),
                    h_r[:, p0 * H2:(j + 1) * H2])

        nc.vector.tensor_copy(h_state[:], h_r[:, (chunk - 1) * H2: chunk * H2])


# ---------------------------------------------------------------------------
# host side
# ---------------------------------------------------------------------------

def prepare_in_maps(x, h0, Wbb, bbb, Wff1, bff1, Wff2, bff2, Wta, bta, Wtb, btb,
                    seq_len=L):
    f32 = np.float32
    Wbb = np.asarray(Wbb, f32)
    W4 = 1.7159 * np.concatenate(
        [np.asarray(Wtb, f32) - np.asarray(Wta, f32),
         np.asarray(Wff1, f32), np.asarray(Wff2, f32)], axis=1)  # (256, 768)
    # w4 sbuf layout: [k0 | k1] along cols
    w4_host = np.concatenate([W4[0:128, :], W4[128:256, :]], axis=1).astype(BF16)
    wbbx_host = Wbb[0:128, :].astype(BF16)                      # (128, 256)
    wbbh_host = np.concatenate([Wbb[128:256, :], Wbb[256:384, :]], axis=1).astype(BF16)

    bbb = np.asarray(bbb, f32); bff1 = np.asarray(bff1, f32)
    bff2 = np.asarray(bff2, f32)
    db = np.asarray(btb, f32) - np.asarray(bta, f32)
    bz_host = np.stack([bbb[0:128], bbb[128:256]]).astype(BF16)          # (2,128)
    bff_host = np.stack([bff1[0:128], bff1[128:256],
                         bff2[0:128], bff2[128:256]]).astype(BF16)       # (4,128)
    bd_host = np.stack([db[0:128], db[128:256]]).astype(BF16)            # (2,128)

    s2_host = np.zeros((2, H2), f32)
    s2_host[0, 0:BC] = 1.0
    s2_host[1, BC:H2] = 1.0
    s2_host = s2_host.astype(BF16)
    s4_host = np.zeros((4, 2 * H2), f32)
    for g in range(4):
        s4_host[g, g * BC:(g + 1) * BC] = 1.0
    s4_host = s4_host.astype(BF16)

    x = np.asarray(x, f32)
    h0 = np.asarray(h0, f32)
    in_maps = []
    for i in range(N_CORES):
        xs = x[i * BC:(i + 1) * BC, :seq_len, :]                # (BC, seq, C)
        xT_host = np.ascontiguousarray(xs.transpose(2, 1, 0)).astype(BF16)
        h0s = h0[i * BC:(i + 1) * BC]                           # (BC, U)
        h0T_host = np.concatenate(
            [h0s[:, 0:128].T, h0s[:, 128:256].T], axis=1).astype(BF16)  # (128, 64)
        in_maps.append({
            "xT": xT_host, "h0T": h0T_host,
            "wbbx": wbbx_host, "wbbh": wbbh_host, "w4": w4_host,
            "bias_z": bz_host, "bias_ff": bff_host, "bias_d": bd_host,
            "sel2": s2_host, "sel4": s4_host,
        })
    return in_maps


def assemble_output(results, seq_len=L):
    """results: list of {'out': (128, seq, 64) bf16} per core -> (B, seq, U) f32."""
    full = np.empty((B, seq_len, U), np.float32)
    for i, r in enumerate(results):
        o = np.asarray(r["out"]).astype(np.float32)      # (128, seq, 64)
        o = o.reshape(128, seq_len, 2, BC)               # p, t, g, b
        full[i * BC:(i + 1) * BC] = o.transpose(3, 1, 2, 0).reshape(BC, seq_len, U)
    return full


_NC_CACHE = {}


def _get_nc(seq_len=L, chunk=256):
    key = (seq_len, chunk)
    if key not in _NC_CACHE:
        _NC_CACHE[key] = build_nc(seq_len=seq_len, chunk=chunk)
    return _NC_CACHE[key]


def kernel(x, h0, Wbb, bbb, Wff1, bff1, Wff2, bff2, Wta, bta, Wtb, btb,
           _trace=False, _seq_len=L, _chunk=256):
    nc = _get_nc(_seq_len, _chunk)
    in_maps = prepare_in_maps(x, h0, Wbb, bbb, Wff1, bff1, Wff2, bff2,
                              Wta, bta, Wtb, btb, seq_len=_seq_len)
    res = bass_utils.run_bass_kernel_spmd(
        nc, in_maps, core_ids=list(range(N_CORES)), trace=_trace)
    readout = assemble_output(res.results, seq_len=_seq_len)
    hN = readout[:, -1, :].copy()
    kernel._last_results = res
    return readout, hN
